# revision 10
# baseline (speedup 1.0000x reference)
"""Distributed causal attention layer for 8 TRN2 NeuronCores.

Problem: payload [4, 2048, 1024], w_qkv [1024, 3072], w_out [1024, 1024],
b_out [1024] -> causal 16-head attention -> out [4, 2048, 1024], f32.

Sharding: core c handles batch b = c//2 and head-half half = c%2 (8 of 16
heads). Attention is fully local per (batch, head-half); the only cross-core
dependency is the output projection (contracts over all 16 heads), resolved
with pair-wise AllGathers of the normalized context (bf16), after which each
core computes its 512 output columns.

Device dataflow (per core, bf16 matmuls, f32 PSUM accumulation), optimized
for PE warmth (HAM clock gate) + engine overlap:
  - Head-PAIR processing: scores for heads A/B are emitted back-to-back with
    lhsT base partitions 0/64 (K=DH=64 -> 64x128 row tiling, T0/T8) so they
    run CONCURRENTLY on the PE array, writing halves of one [128,1024] PSUM
    tile; ONE 1024-wide exp per k-tile on ScalarE (scale=1/8 fused).
  - Diagonal k-tiles: fused [128,2x128] triangular-mask add on VectorE; the
    scores matmul rhs, the exp AP and the ctx matmul rhs are all NARROWED to
    the causal range (no memsets, no wasted exp/PE columns).
  - ctx'T[65, q] = V'^T @ E accumulated over k per head (ones-augmented V
    gives softmax denominators in row 64); normalization via two concurrent
    col-tiled bf16 K=1 broadcast matmuls (positions (0,0)/(0,64)) +
    reciprocal + 2 multiplies. NOTE: an fp32 K=1 broadcast matmul variant
    hangs the PE - keep it bf16.
  - Software pipeline per (head-pair, q-chunk): scores(t) || exp(t-1) ||
    ctx(t-2), with V'-build groups, the next head-pair's Q/K projection
    chunks and out-projection installments INJECTED into the ACT-bound
    attention regions so the PE queue never drains (keeps HAM at 2.4 GHz).
  - Out-projection runs in 4 installments (one per gathered chunk pair)
    accumulating into SBUF f32; hp3's AllGather is split into q-halves so
    only the last 8 row-tiles wait on the final 256KB gather.
"""

import os
import numpy as np
import ml_dtypes

import concourse.bass as bass
import concourse.bacc as bacc
import concourse.mybir as mybir
import concourse.tile as tile
from concourse import bass_utils

B, S, D, H = 4, 2048, 1024, 16
DH = 64
HL = H // 2          # 8 local heads per core
DL = HL * DH         # 512 local head channels
N_CORES = 8
P = 128              # partitions
KC = D // P          # 8 contraction chunks for D
NEG = -1.0e9

BF16 = mybir.dt.bfloat16
F32 = mybir.dt.float32

LAST_EXEC_TIME_NS = None
_CACHED_NC = None


def _build():
    nc = bacc.Bacc(None, target_bir_lowering=False, debug=False)

    pt_d = nc.declare_dram_parameter("pt", [D, S], BF16, isOutput=False)
    wq_d = nc.declare_dram_parameter("wq", [D, DL], BF16, isOutput=False)
    wk_d = nc.declare_dram_parameter("wk", [D, DL], BF16, isOutput=False)
    wv_d = nc.declare_dram_parameter("wv", [D, DL], BF16, isOutput=False)
    wo_d = nc.declare_dram_parameter("wo", [D, DL], BF16, isOutput=False)
    bo_d = nc.declare_dram_parameter("bo", [1, DL], BF16, isOutput=False)
    out_d = nc.declare_dram_parameter("out", [S, DL], F32, isOutput=True)

    # upper-triangular causal mask block tri[p, c] = 0 if p <= c else NEG,
    # duplicated side-by-side so one DVE add masks both heads' halves
    tri_np = np.where(
        np.arange(P)[:, None] <= np.arange(P)[None, :], 0.0, NEG
    ).astype(np.float32)
    tri2_d = nc.inline_tensor(
        np.concatenate([tri_np, tri_np], axis=1), name="tri2_const"
    )

    skip_cc = os.environ.get("KERNEL_SKIP_CC") == "1"

    with tile.TileContext(nc) as tc:
        with (
            tc.tile_pool(name="weights", bufs=1) as wpool,
            tc.tile_pool(name="payload", bufs=1) as ppool,
            tc.tile_pool(name="qk", bufs=1) as qkpool,
            tc.tile_pool(name="vp", bufs=1) as vpool,
            tc.tile_pool(name="et", bufs=4) as etpool,
            tc.tile_pool(name="ctxn", bufs=1) as cnpool,
            tc.tile_pool(name="ctxf", bufs=1) as cfpool,
            tc.tile_pool(name="soacc", bufs=1) as sopool,
            tc.tile_pool(name="small", bufs=4) as smpool,
            tc.tile_pool(name="rbcp", bufs=2) as rbcpool,
            tc.tile_pool(name="sp", bufs=2, space="PSUM") as sppool,
            tc.tile_pool(name="cps", bufs=1, space="PSUM") as cpool,
            tc.tile_pool(name="mm", bufs=2, space="PSUM") as mmpool,
            tc.tile_pool(name="dram", bufs=1, space="DRAM") as dpool,
        ):
            # ---- input DMAs (kc-interleaved so proj MMs can chase them) ----
            pt_sb, wq_sb, wk_sb, wv_sb, wo_sb = [], [], [], [], []
            for kc in range(KC):
                sl = slice(kc * P, (kc + 1) * P)
                t = ppool.tile([P, S], BF16, tag=f"pt{kc}", name="pt_sb")
                nc.sync.dma_start(out=t[:, :], in_=pt_d[sl, :])
                pt_sb.append(t)
                for lst, dram, wname in ((wk_sb, wk_d, "wk"), (wq_sb, wq_d, "wq"),
                                         (wv_sb, wv_d, "wv")):
                    w = wpool.tile([P, DL], BF16, tag=f"{wname}{kc}", name="w_sb")
                    nc.sync.dma_start(out=w[:, :], in_=dram[sl, :])
                    lst.append(w)
            for kc in range(KC):
                w = wpool.tile([P, DL], BF16, tag=f"wo{kc}", name="w_sb")
                nc.sync.dma_start(out=w[:, :], in_=wo_d[kc * P:(kc + 1) * P, :])
                wo_sb.append(w)
            bo_sb = wpool.tile([1, DL], BF16, tag="bo")
            nc.sync.dma_start(out=bo_sb[:, :], in_=bo_d[:, :])
            tri2_sb = wpool.tile([P, 2 * P], F32, tag="tri2")
            nc.sync.dma_start(out=tri2_sb[:, :], in_=tri2_d[:, :])

            ones_bf = wpool.tile([1, P], BF16, tag="ones_bf")
            nc.vector.memset(ones_bf[:, :], 1.0)

            tri2_3 = tri2_sb.rearrange("p (h w) -> p h w", h=2)

            # ---- persistent SBUF tiles ----
            vp_sb = [None] * 16          # V' [128, 520] per k-tile
            qk_tiles = {}                # hp -> (kt, qt) [128, S] bf16
            ctxn_sb = [cnpool.tile([P, S], BF16, tag=f"ctxn{i}", name="ctxn")
                       for i in range(4)]
            ctxf_sb = [cfpool.tile([P, S], BF16, tag=f"ctxf{i}", name="ctxf_sb")
                       for i in range(KC)]
            so_acc = [sopool.tile([P, DL], F32, tag=f"soa{i}", name="so_acc")
                      for i in range(16)]

            cc_in = [dpool.tile([P, S], BF16, tag=f"ccin{i}", name="cc_in")
                     for i in range(3)]
            cc_out = [dpool.tile([2 * P, S], BF16, tag=f"ccout{i}", name="cc_out")
                      for i in range(3)]
            # hp3's gather is split by q-range: [0:1024), [1024:1536), [1536:2048)
            G3 = [(0, 1024), (1024, 1536), (1536, 2048)]
            cc_in3 = [dpool.tile([P, g1 - g0], BF16, tag=f"ccin3{i}",
                                 name="cc_in3") for i, (g0, g1) in enumerate(G3)]
            cc_out3 = [dpool.tile([2 * P, g1 - g0], BF16, tag=f"ccout3{i}",
                                  name="cc_out3") for i, (g0, g1) in enumerate(G3)]

            # ---- work units ----
            def proj_unit(hp, nj):
                """Q/K projection for head-pair hp, q-chunk nj (16 MMs)."""
                if nj == 0:
                    kt = qkpool.tile([P, S], BF16, tag=f"kt{hp % 2}", name="kt")
                    qt = qkpool.tile([P, S], BF16, tag=f"qt{hp % 2}", name="qt")
                    qk_tiles[hp] = (kt, qt)
                kt, qt = qk_tiles[hp]
                for w_sb, dst in ((wk_sb, kt), (wq_sb, qt)):
                    ps = mmpool.tile([P, DL], F32, tag="mm", name="ps")
                    for kc in range(KC):
                        nc.tensor.matmul(
                            ps[:, :],
                            lhsT=w_sb[kc][:, hp * P:(hp + 1) * P],
                            rhs=pt_sb[kc][:, nj * DL:(nj + 1) * DL],
                            start=(kc == 0),
                            stop=(kc == KC - 1),
                        )
                    nc.vector.tensor_copy(
                        out=dst[:, nj * DL:(nj + 1) * DL], in_=ps[:, :]
                    )

            def v_unit(g):
                """V' build for k-tiles 4g..4g+3 (32 MMs + ACT copies)."""
                for st in range(4 * g, 4 * g + 4):
                    vt = vpool.tile([P, HL * (DH + 1)], BF16, tag=f"vp{st}",
                                    name="vp_sb")
                    ps = mmpool.tile([P, DL], F32, tag="mm", name="ps")
                    for kc in range(KC):
                        nc.tensor.matmul(
                            ps[:, :],
                            lhsT=pt_sb[kc][:, st * P:(st + 1) * P],
                            rhs=wv_sb[kc][:, :],
                            start=(kc == 0),
                            stop=(kc == KC - 1),
                        )
                    nc.vector.memset(
                        vt.rearrange("p (h c) -> p h c", h=HL)[:, :, DH:DH + 1],
                        1.0,
                    )
                    nc.vector.tensor_copy(
                        out=vt.rearrange("p (h c) -> p h c", h=HL)[:, :, 0:DH],
                        in_=ps.rearrange("p (h c) -> p h c", h=HL)[:, :, :],
                    )
                    vp_sb[st] = vt

            def o_unit(pair, mts):
                """Out-proj installment: gathered chunk pair {2p, 2p+1},
                row-tiles mts, accumulated into so_acc. For the last pair the
                bias K=1 ones matmul OPENS the group (it has no gather
                dependency, so it runs during the gather wait)."""
                for mt in mts:
                    ps = mmpool.tile([P, DL], F32, tag="mm", name="ps")
                    if pair == 3:
                        nc.tensor.matmul(
                            ps[:, :], lhsT=ones_bf[0:1, :], rhs=bo_sb[0:1, :],
                            start=True, stop=False,
                        )
                    for i, kc in enumerate((2 * pair, 2 * pair + 1)):
                        nc.tensor.matmul(
                            ps[:, :],
                            lhsT=ctxf_sb[kc][:, mt * P:(mt + 1) * P],
                            rhs=wo_sb[kc][:, :],
                            start=(i == 0 and pair != 3),
                            stop=(i == 1),
                        )
                    if pair == 0:
                        nc.vector.tensor_copy(out=so_acc[mt][:, :], in_=ps[:, :])
                    else:
                        nc.vector.tensor_add(
                            so_acc[mt][:, :], so_acc[mt][:, :], ps[:, :]
                        )
                    if pair == 3:
                        nc.sync.dma_start(
                            out=out_d[mt * P:(mt + 1) * P, :],
                            in_=so_acc[mt][:, :],
                        )

            def norm(hp, j, cpsA, cpsB):
                """Normalize both heads' ctx chunk j into ctxn[hp]."""
                sums = []
                for tag, cps in (("sumsA", cpsA), ("sumsB", cpsB)):
                    s = smpool.tile([1, DL], BF16, tag=tag, name="sums")
                    nc.vector.tensor_copy(out=s[:, :], in_=cps[DH:DH + 1, :])
                    sums.append(s)
                # two concurrent col-tiled K=1 broadcast matmuls (bf16!)
                bps = mmpool.tile([P, DL], F32, tag="mm", name="bps")
                nc.tensor.matmul(bps[0:DH, :], lhsT=ones_bf[0:1, 0:DH],
                                 rhs=sums[0][0:1, :], start=True, stop=True)
                nc.tensor.matmul(bps[DH:2 * DH, :], lhsT=ones_bf[0:1, 0:DH],
                                 rhs=sums[1][0:1, :], start=True, stop=True)
                rbc = rbcpool.tile([P, DL], F32, tag="rbc", name="rbc")
                nc.vector.reciprocal_approx_fast(rbc[:, :], bps[:, :])
                jc = slice(j * DL, (j + 1) * DL)
                nc.vector.tensor_mul(
                    ctxn_sb[hp][0:DH, jc], cpsA[0:DH, :], rbc[0:DH, :])
                nc.vector.tensor_mul(
                    ctxn_sb[hp][DH:2 * DH, jc], cpsB[0:DH, :], rbc[DH:2 * DH, :])

            def attention(hp, j):
                """Causal attention for head pair hp over q-chunk j.
                Pipeline: scores(t) || exp(t-1) || ctx(t-2)."""
                kt, qt = qk_tiles[hp]
                nt = 4 * j + 4
                cpsA = cpool.tile([P, DL], F32, tag="cpsA", name="cpsA")
                cpsB = cpool.tile([P, DL], F32, tag="cpsB", name="cpsB")
                ets = {}
                offs = {}

                def ctx_emit(i):
                    off = offs[i]
                    et3 = ets[i].rearrange("p (h w) -> p h w", h=2)
                    for hh, cps in ((0, cpsA), (1, cpsB)):
                        h = 2 * hp + hh
                        nc.tensor.matmul(
                            cps[0:DH + 1, off:DL],
                            lhsT=vp_sb[i][:, h * (DH + 1):(h + 1) * (DH + 1)],
                            rhs=et3[:, hh, off:DL],
                            start=(i == 0),
                            stop=(i == nt - 1),
                        )

                def scores_emit(t):
                    diag = t >= 4 * j
                    off = (t - 4 * j) * P if diag else 0
                    offs[t] = off
                    sp = sppool.tile([P, 2 * DL], F32, tag="sp", name="sp")
                    sp3 = sp.rearrange("p (h w) -> p h w", h=2)
                    for hh in range(2):
                        pb = hh * DH
                        nc.tensor.matmul(
                            sp3[:, hh, off:DL],
                            lhsT=kt[pb:pb + DH, t * P:(t + 1) * P],
                            rhs=qt[pb:pb + DH, j * DL + off:(j + 1) * DL],
                            start=True,
                            stop=True,
                        )
                    if diag:
                        nc.vector.tensor_add(
                            sp3[:, :, off:off + P],
                            sp3[:, :, off:off + P],
                            tri2_3[:, :, :],
                        )
                    et = etpool.tile([P, 2 * DL], BF16, tag="et", name="et")
                    ets[t] = et
                    if off:
                        nc.scalar.activation(
                            et.rearrange("p (h w) -> p h w", h=2)[:, :, off:DL],
                            sp3[:, :, off:DL],
                            mybir.ActivationFunctionType.Exp,
                            scale=0.125,
                        )
                    else:
                        nc.scalar.activation(
                            et[:, :], sp[:, :],
                            mybir.ActivationFunctionType.Exp,
                            scale=0.125,
                        )

                # t-PAIR batched pipeline: [S(2p) S(2p+1)] in 64-row mode,
                # then [C(2p-2) C(2p-1)] in 128-row mode — halves the PE
                # tiling-mode switches vs per-t alternation.
                for p in range(nt // 2):
                    scores_emit(2 * p)
                    scores_emit(2 * p + 1)
                    if p >= 1:
                        ctx_emit(2 * p - 2)
                        ctx_emit(2 * p - 1)
                ctx_emit(nt - 2)
                ctx_emit(nt - 1)
                norm(hp, j, cpsA, cpsB)

            def gather(hp):
                """Pair AllGather of ctxn[hp] -> ctxf chunks 2hp, 2hp+1."""
                nc.sync.dma_start(out=cc_in[hp][:, :], in_=ctxn_sb[hp][:, :])
                if skip_cc:
                    nc.sync.dma_start(out=cc_out[hp][0:P, :], in_=cc_in[hp][:, :])
                    nc.sync.dma_start(out=cc_out[hp][P:2 * P, :],
                                      in_=cc_in[hp][:, :])
                else:
                    nc.gpsimd.collective_compute(
                        "AllGather",
                        mybir.AluOpType.bypass,
                        replica_groups=[[0, 1], [2, 3], [4, 5], [6, 7]],
                        ins=[cc_in[hp].opt()],
                        outs=[cc_out[hp].opt()],
                    )
                nc.sync.dma_start(out=ctxf_sb[2 * hp][:, :],
                                  in_=cc_out[hp][0:P, :])
                nc.sync.dma_start(out=ctxf_sb[2 * hp + 1][:, :],
                                  in_=cc_out[hp][P:2 * P, :])

            def gather3(g):
                """hp3's AllGather, q-range piece g."""
                g0, g1 = G3[g]
                qc = slice(g0, g1)
                nc.sync.dma_start(out=cc_in3[g][:, :], in_=ctxn_sb[3][:, qc])
                if skip_cc:
                    nc.sync.dma_start(out=cc_out3[g][0:P, :],
                                      in_=cc_in3[g][:, :])
                    nc.sync.dma_start(out=cc_out3[g][P:2 * P, :],
                                      in_=cc_in3[g][:, :])
                else:
                    nc.gpsimd.collective_compute(
                        "AllGather",
                        mybir.AluOpType.bypass,
                        replica_groups=[[0, 1], [2, 3], [4, 5], [6, 7]],
                        ins=[cc_in3[g].opt()],
                        outs=[cc_out3[g].opt()],
                    )
                nc.sync.dma_start(out=ctxf_sb[6][:, qc], in_=cc_out3[g][0:P, :])
                nc.sync.dma_start(out=ctxf_sb[7][:, qc],
                                  in_=cc_out3[g][P:2 * P, :])

            # ---- emission schedule (program order ~ per-engine order) ----
            proj_unit(0, 0)
            v_unit(0)
            attention(0, 0)
            for j in (1, 2, 3):
                proj_unit(0, j)
                v_unit(j)
                attention(0, j)
            gather(0)
            proj_unit(1, 0)

            # o_units are stamped with tile_wait_until so the scheduler's
            # stage-1A sim (which models collectives as ~free) cannot hoist
            # them onto the PE queue ahead of ready attention work.
            attention(1, 0)
            proj_unit(1, 1)
            attention(1, 1)
            proj_unit(1, 2)
            with tc.tile_wait_until(0.200):
                o_unit(0, (0, 1, 2, 3, 4, 5, 6, 7))
            attention(1, 2)
            proj_unit(1, 3)
            with tc.tile_wait_until(0.215):
                o_unit(0, (8, 9, 10, 11, 12, 13, 14, 15))
            attention(1, 3)
            gather(1)
            proj_unit(2, 0)

            attention(2, 0)
            proj_unit(2, 1)
            attention(2, 1)
            proj_unit(2, 2)
            with tc.tile_wait_until(0.265):
                o_unit(1, (0, 1, 2, 3, 4, 5, 6, 7))
            attention(2, 2)
            proj_unit(2, 3)
            with tc.tile_wait_until(0.280):
                o_unit(1, (8, 9, 10, 11, 12, 13, 14, 15))
            attention(2, 3)
            gather(2)
            proj_unit(3, 0)

            attention(3, 0)
            proj_unit(3, 1)
            attention(3, 1)
            proj_unit(3, 2)
            gather3(0)
            with tc.tile_wait_until(0.330):
                o_unit(2, (0, 1, 2, 3, 4, 5, 6, 7))
            attention(3, 2)
            proj_unit(3, 3)
            gather3(1)
            with tc.tile_wait_until(0.345):
                o_unit(2, (8, 9, 10, 11, 12, 13, 14, 15))
            attention(3, 3)
            gather3(2)
            with tc.tile_wait_until(0.360):
                o_unit(3, (0, 1, 2, 3, 4, 5, 6, 7))   # cols [0:1024] via g3(0)
            with tc.tile_wait_until(0.370):
                o_unit(3, (8, 9, 10, 11))             # g3(1)
            with tc.tile_wait_until(0.380):
                o_unit(3, (12, 13, 14, 15))           # g3(2)

    nc.finalize()
    return nc


def kernel(payload, w_qkv, w_out, b_out):
    global LAST_EXEC_TIME_NS, _CACHED_NC
    payload = np.asarray(payload, dtype=np.float32)
    w_qkv = np.asarray(w_qkv, dtype=np.float32)
    w_out = np.asarray(w_out, dtype=np.float32)
    b_out = np.asarray(b_out, dtype=np.float32)

    bf = ml_dtypes.bfloat16
    # w_out rows permuted to match gathered ctx chunk order:
    # chunk 2*hp   = even core's head-pair hp -> rows [128hp, 128hp+128)
    # chunk 2*hp+1 = odd  core's head-pair hp -> rows [512+128hp, ...)
    row_perm = np.concatenate(
        [np.r_[128 * hp:128 * hp + 128, 512 + 128 * hp:512 + 128 * hp + 128]
         for hp in range(4)]
    )
    w_out_p = w_out[row_perm]

    in_maps = []
    for c in range(N_CORES):
        b, half = c // 2, c % 2
        cols = slice(half * DL, (half + 1) * DL)
        in_maps.append({
            "pt": np.ascontiguousarray(payload[b].T).astype(bf),
            "wq": np.ascontiguousarray(w_qkv[:, cols]).astype(bf),
            "wk": np.ascontiguousarray(w_qkv[:, D:][:, cols]).astype(bf),
            "wv": np.ascontiguousarray(w_qkv[:, 2 * D:][:, cols]).astype(bf),
            "wo": np.ascontiguousarray(w_out_p[:, cols]).astype(bf),
            "bo": np.ascontiguousarray(b_out[cols]).reshape(1, DL).astype(bf),
        })

    if _CACHED_NC is None:
        _CACHED_NC = _build()
    res = bass_utils.run_bass_kernel_spmd(
        _CACHED_NC, in_maps, core_ids=list(range(N_CORES))
    )
    LAST_EXEC_TIME_NS = res.exec_time_ns

    out = np.empty((B, S, D), dtype=np.float32)
    for c in range(N_CORES):
        b, half = c // 2, c % 2
        out[b, :, half * DL:(half + 1) * DL] = res.results[c]["out"]
    return out


# revision 12
# speedup vs baseline: 1.1816x; 1.1816x over previous
"""Distributed causal attention layer for 8 TRN2 NeuronCores.

Problem: payload [4, 2048, 1024], w_qkv [1024, 3072], w_out [1024, 1024],
b_out [1024] -> causal 16-head attention -> out [4, 2048, 1024], f32.

Sharding: core c handles batch b = c//2 and head-half half = c%2 (8 of 16
heads). Attention is fully local per (batch, head-half); the only cross-core
dependency is the output projection (contracts over all 16 heads), resolved
with pair-wise AllGathers of the normalized context (bf16), after which each
core computes its 512 output columns.

Device dataflow (per core, bf16 matmuls, f32 PSUM accumulation), optimized
for PE warmth (HAM clock gate) + engine overlap:
  - Head-PAIR processing: scores for heads A/B are emitted back-to-back with
    lhsT base partitions 0/64 (K=DH=64 -> 64x128 row tiling, T0/T8) so they
    run CONCURRENTLY on the PE array, writing halves of one [128,1024] PSUM
    tile; ONE 1024-wide exp per k-tile on ScalarE (scale=1/8 fused).
  - Diagonal k-tiles: fused [128,2x128] triangular-mask add on VectorE; the
    scores matmul rhs, the exp AP and the ctx matmul rhs are all NARROWED to
    the causal range (no memsets, no wasted exp/PE columns).
  - ctx'T[65, q] = V'^T @ E accumulated over k per head (ones-augmented V
    gives softmax denominators in row 64); normalization via two concurrent
    col-tiled bf16 K=1 broadcast matmuls (positions (0,0)/(0,64)) +
    reciprocal + 2 multiplies. NOTE: an fp32 K=1 broadcast matmul variant
    hangs the PE - keep it bf16.
  - Software pipeline per (head-pair, q-chunk): scores(t) || exp(t-1) ||
    ctx(t-2), with V'-build groups, the next head-pair's Q/K projection
    chunks and out-projection installments INJECTED into the ACT-bound
    attention regions so the PE queue never drains (keeps HAM at 2.4 GHz).
  - Out-projection runs in 4 installments (one per gathered chunk pair)
    accumulating into SBUF f32; hp3's AllGather is split into q-halves so
    only the last 8 row-tiles wait on the final 256KB gather.
"""

import os
import numpy as np
import ml_dtypes

import concourse.bass as bass
import concourse.bacc as bacc
import concourse.mybir as mybir
import concourse.tile as tile
from concourse import bass_utils

B, S, D, H = 4, 2048, 1024, 16
DH = 64
HL = H // 2          # 8 local heads per core
DL = HL * DH         # 512 local head channels
N_CORES = 8
P = 128              # partitions
KC = D // P          # 8 contraction chunks for D
NEG = -1.0e9

BF16 = mybir.dt.bfloat16
F32 = mybir.dt.float32

LAST_EXEC_TIME_NS = None
_CACHED_NC = None


def _build():
    nc = bacc.Bacc(None, target_bir_lowering=False, debug=False)

    pt_d = nc.declare_dram_parameter("pt", [D, S], BF16, isOutput=False)
    wq_d = nc.declare_dram_parameter("wq", [D, DL], BF16, isOutput=False)
    wk_d = nc.declare_dram_parameter("wk", [D, DL], BF16, isOutput=False)
    wv_d = nc.declare_dram_parameter("wv", [D, DL], BF16, isOutput=False)
    wo_d = nc.declare_dram_parameter("wo", [D, DL], BF16, isOutput=False)
    bo_d = nc.declare_dram_parameter("bo", [1, DL], BF16, isOutput=False)
    out_d = nc.declare_dram_parameter("out", [S, DL], F32, isOutput=True)

    # upper-triangular causal mask block tri[p, c] = 0 if p <= c else NEG,
    # duplicated side-by-side so one DVE add masks both heads' halves
    tri_np = np.where(
        np.arange(P)[:, None] <= np.arange(P)[None, :], 0.0, NEG
    ).astype(np.float32)
    tri2_d = nc.inline_tensor(
        np.concatenate([tri_np, tri_np], axis=1), name="tri2_const"
    )

    skip_cc = os.environ.get("KERNEL_SKIP_CC") == "1"

    with tile.TileContext(nc) as tc:
        with (
            tc.tile_pool(name="weights", bufs=1) as wpool,
            tc.tile_pool(name="payload", bufs=1) as ppool,
            tc.tile_pool(name="qk", bufs=1) as qkpool,
            tc.tile_pool(name="vp", bufs=1) as vpool,
            tc.tile_pool(name="et", bufs=4) as etpool,
            tc.tile_pool(name="ctxn", bufs=1) as cnpool,
            tc.tile_pool(name="ctxf", bufs=1) as cfpool,
            tc.tile_pool(name="soacc", bufs=1) as sopool,
            tc.tile_pool(name="small", bufs=4) as smpool,
            tc.tile_pool(name="rbcp", bufs=2) as rbcpool,
            tc.tile_pool(name="sp", bufs=2, space="PSUM") as sppool,
            tc.tile_pool(name="cps", bufs=1, space="PSUM") as cpool,
            tc.tile_pool(name="mm", bufs=2, space="PSUM") as mmpool,
            tc.tile_pool(name="dram", bufs=1, space="DRAM") as dpool,
        ):
            # ---- input DMAs (kc-interleaved so proj MMs can chase them) ----
            pt_sb, wq_sb, wk_sb, wv_sb, wo_sb = [], [], [], [], []
            for kc in range(KC):
                sl = slice(kc * P, (kc + 1) * P)
                t = ppool.tile([P, S], BF16, tag=f"pt{kc}", name="pt_sb")
                nc.sync.dma_start(out=t[:, :], in_=pt_d[sl, :])
                pt_sb.append(t)
                for lst, dram, wname in ((wk_sb, wk_d, "wk"), (wq_sb, wq_d, "wq")):
                    w = wpool.tile([P, DL], BF16, tag=f"{wname}{kc}", name="w_sb")
                    nc.sync.dma_start(out=w[:, :], in_=dram[sl, :])
                    lst.append(w)
            for kc in range(KC):
                sl = slice(kc * P, (kc + 1) * P)
                w = wpool.tile([P, DL], BF16, tag=f"wv{kc}", name="w_sb")
                nc.sync.dma_start(out=w[:, :], in_=wv_d[sl, :])
                wv_sb.append(w)
            for kc in range(KC):
                w = wpool.tile([P, DL], BF16, tag=f"wo{kc}", name="w_sb")
                nc.sync.dma_start(out=w[:, :], in_=wo_d[kc * P:(kc + 1) * P, :])
                wo_sb.append(w)
            bo_sb = wpool.tile([1, DL], BF16, tag="bo")
            nc.sync.dma_start(out=bo_sb[:, :], in_=bo_d[:, :])
            tri2_sb = wpool.tile([P, 2 * P], F32, tag="tri2")
            nc.sync.dma_start(out=tri2_sb[:, :], in_=tri2_d[:, :])

            ones_bf = wpool.tile([1, P], BF16, tag="ones_bf")
            nc.vector.memset(ones_bf[:, :], 1.0)

            tri2_3 = tri2_sb.rearrange("p (h w) -> p h w", h=2)

            # ---- persistent SBUF tiles ----
            vp_sb = [None] * 16          # V' [128, 520] per k-tile
            qk_tiles = {}                # hp -> (kt, qt) [128, S] bf16
            ctxn_sb = [cnpool.tile([P, S], BF16, tag=f"ctxn{i}", name="ctxn")
                       for i in range(4)]
            ctxf_sb = [cfpool.tile([P, S], BF16, tag=f"ctxf{i}", name="ctxf_sb")
                       for i in range(KC)]
            so_acc = [sopool.tile([P, DL], F32, tag=f"soa{i}", name="so_acc")
                      for i in range(16)]

            cc_in = [dpool.tile([P, S], BF16, tag=f"ccin{i}", name="cc_in")
                     for i in range(3)]
            cc_out = [dpool.tile([2 * P, S], BF16, tag=f"ccout{i}", name="cc_out")
                      for i in range(3)]
            # hp3's gather is split by q-range: [0:1024), [1024:1536), [1536:2048)
            G3 = [(0, 1024), (1024, 1536), (1536, 2048)]
            cc_in3 = [dpool.tile([P, g1 - g0], BF16, tag=f"ccin3{i}",
                                 name="cc_in3") for i, (g0, g1) in enumerate(G3)]
            cc_out3 = [dpool.tile([2 * P, g1 - g0], BF16, tag=f"ccout3{i}",
                                  name="cc_out3") for i, (g0, g1) in enumerate(G3)]

            # ---- work units ----
            def proj_unit(hp, nj):
                """Q/K projection for head-pair hp, q-chunk nj (16 MMs)."""
                if nj == 0:
                    kt = qkpool.tile([P, S], BF16, tag=f"kt{hp % 2}", name="kt")
                    qt = qkpool.tile([P, S], BF16, tag=f"qt{hp % 2}", name="qt")
                    qk_tiles[hp] = (kt, qt)
                kt, qt = qk_tiles[hp]
                for w_sb, dst in ((wk_sb, kt), (wq_sb, qt)):
                    ps = mmpool.tile([P, DL], F32, tag="mm", name="ps")
                    for kc in range(KC):
                        nc.tensor.matmul(
                            ps[:, :],
                            lhsT=w_sb[kc][:, hp * P:(hp + 1) * P],
                            rhs=pt_sb[kc][:, nj * DL:(nj + 1) * DL],
                            start=(kc == 0),
                            stop=(kc == KC - 1),
                        )
                    nc.vector.tensor_copy(
                        out=dst[:, nj * DL:(nj + 1) * DL], in_=ps[:, :]
                    )

            def v_unit(g):
                """V' build for k-tiles 4g..4g+3 (32 MMs + ACT copies)."""
                for st in range(4 * g, 4 * g + 4):
                    vt = vpool.tile([P, HL * (DH + 1)], BF16, tag=f"vp{st}",
                                    name="vp_sb")
                    ps = mmpool.tile([P, DL], F32, tag="mm", name="ps")
                    for kc in range(KC):
                        nc.tensor.matmul(
                            ps[:, :],
                            lhsT=pt_sb[kc][:, st * P:(st + 1) * P],
                            rhs=wv_sb[kc][:, :],
                            start=(kc == 0),
                            stop=(kc == KC - 1),
                        )
                    nc.vector.memset(
                        vt.rearrange("p (h c) -> p h c", h=HL)[:, :, DH:DH + 1],
                        1.0,
                    )
                    nc.vector.tensor_copy(
                        out=vt.rearrange("p (h c) -> p h c", h=HL)[:, :, 0:DH],
                        in_=ps.rearrange("p (h c) -> p h c", h=HL)[:, :, :],
                    )
                    vp_sb[st] = vt

            def o_unit(pair, mts):
                """Out-proj installment: gathered chunk pair {2p, 2p+1},
                row-tiles mts, accumulated into so_acc. For the last pair the
                bias K=1 ones matmul OPENS the group (it has no gather
                dependency, so it runs during the gather wait)."""
                for mt in mts:
                    ps = mmpool.tile([P, DL], F32, tag="mm", name="ps")
                    if pair == 3:
                        nc.tensor.matmul(
                            ps[:, :], lhsT=ones_bf[0:1, :], rhs=bo_sb[0:1, :],
                            start=True, stop=False,
                        )
                    for i, kc in enumerate((2 * pair, 2 * pair + 1)):
                        nc.tensor.matmul(
                            ps[:, :],
                            lhsT=ctxf_sb[kc][:, mt * P:(mt + 1) * P],
                            rhs=wo_sb[kc][:, :],
                            start=(i == 0 and pair != 3),
                            stop=(i == 1),
                        )
                    if pair == 0:
                        nc.vector.tensor_copy(out=so_acc[mt][:, :], in_=ps[:, :])
                    else:
                        nc.vector.tensor_add(
                            so_acc[mt][:, :], so_acc[mt][:, :], ps[:, :]
                        )
                    if pair == 3:
                        nc.sync.dma_start(
                            out=out_d[mt * P:(mt + 1) * P, :],
                            in_=so_acc[mt][:, :],
                        )

            def norm(hp, j, cpsA, cpsB):
                """Normalize both heads' ctx chunk j into ctxn[hp]."""
                sums = []
                for tag, cps in (("sumsA", cpsA), ("sumsB", cpsB)):
                    s = smpool.tile([1, DL], BF16, tag=tag, name="sums")
                    nc.vector.tensor_copy(out=s[:, :], in_=cps[DH:DH + 1, :])
                    sums.append(s)
                # two concurrent col-tiled K=1 broadcast matmuls (bf16!)
                bps = mmpool.tile([P, DL], F32, tag="mm", name="bps")
                nc.tensor.matmul(bps[0:DH, :], lhsT=ones_bf[0:1, 0:DH],
                                 rhs=sums[0][0:1, :], start=True, stop=True)
                nc.tensor.matmul(bps[DH:2 * DH, :], lhsT=ones_bf[0:1, 0:DH],
                                 rhs=sums[1][0:1, :], start=True, stop=True)
                rbc = rbcpool.tile([P, DL], F32, tag="rbc", name="rbc")
                nc.vector.reciprocal_approx_fast(rbc[:, :], bps[:, :])
                jc = slice(j * DL, (j + 1) * DL)
                nc.vector.tensor_mul(
                    ctxn_sb[hp][0:DH, jc], cpsA[0:DH, :], rbc[0:DH, :])
                nc.vector.tensor_mul(
                    ctxn_sb[hp][DH:2 * DH, jc], cpsB[0:DH, :], rbc[DH:2 * DH, :])

            def attention(hp, j):
                """Causal attention for head pair hp over q-chunk j.
                Pipeline: scores(t) || exp(t-1) || ctx(t-2)."""
                kt, qt = qk_tiles[hp]
                nt = 4 * j + 4
                cpsA = cpool.tile([P, DL], F32, tag="cpsA", name="cpsA")
                cpsB = cpool.tile([P, DL], F32, tag="cpsB", name="cpsB")
                ets = {}
                offs = {}

                def ctx_emit(i):
                    off = offs[i]
                    et3 = ets[i].rearrange("p (h w) -> p h w", h=2)
                    for hh, cps in ((0, cpsA), (1, cpsB)):
                        h = 2 * hp + hh
                        nc.tensor.matmul(
                            cps[0:DH + 1, off:DL],
                            lhsT=vp_sb[i][:, h * (DH + 1):(h + 1) * (DH + 1)],
                            rhs=et3[:, hh, off:DL],
                            start=(i == 0),
                            stop=(i == nt - 1),
                        )

                def scores_emit(t):
                    diag = t >= 4 * j
                    off = (t - 4 * j) * P if diag else 0
                    offs[t] = off
                    sp = sppool.tile([P, 2 * DL], F32, tag="sp", name="sp")
                    sp3 = sp.rearrange("p (h w) -> p h w", h=2)
                    for hh in range(2):
                        pb = hh * DH
                        nc.tensor.matmul(
                            sp3[:, hh, off:DL],
                            lhsT=kt[pb:pb + DH, t * P:(t + 1) * P],
                            rhs=qt[pb:pb + DH, j * DL + off:(j + 1) * DL],
                            start=True,
                            stop=True,
                        )
                    if diag:
                        nc.vector.tensor_add(
                            sp3[:, :, off:off + P],
                            sp3[:, :, off:off + P],
                            tri2_3[:, :, :],
                        )
                    et = etpool.tile([P, 2 * DL], BF16, tag="et", name="et")
                    ets[t] = et
                    if off:
                        nc.scalar.activation(
                            et.rearrange("p (h w) -> p h w", h=2)[:, :, off:DL],
                            sp3[:, :, off:DL],
                            mybir.ActivationFunctionType.Exp,
                            scale=0.125,
                        )
                    else:
                        nc.scalar.activation(
                            et[:, :], sp[:, :],
                            mybir.ActivationFunctionType.Exp,
                            scale=0.125,
                        )

                # pipeline: scores(t) || exp(t-1) || ctx(t-2)
                for t in range(nt):
                    scores_emit(t)
                    if t >= 2:
                        ctx_emit(t - 2)
                ctx_emit(nt - 2)
                ctx_emit(nt - 1)
                norm(hp, j, cpsA, cpsB)

            def gather(hp):
                """Pair AllGather of ctxn[hp] -> ctxf chunks 2hp, 2hp+1."""
                nc.sync.dma_start(out=cc_in[hp][:, :], in_=ctxn_sb[hp][:, :])
                if skip_cc:
                    nc.sync.dma_start(out=cc_out[hp][0:P, :], in_=cc_in[hp][:, :])
                    nc.sync.dma_start(out=cc_out[hp][P:2 * P, :],
                                      in_=cc_in[hp][:, :])
                else:
                    nc.gpsimd.collective_compute(
                        "AllGather",
                        mybir.AluOpType.bypass,
                        replica_groups=[[0, 1], [2, 3], [4, 5], [6, 7]],
                        ins=[cc_in[hp].opt()],
                        outs=[cc_out[hp].opt()],
                    )
                nc.sync.dma_start(out=ctxf_sb[2 * hp][:, :],
                                  in_=cc_out[hp][0:P, :])
                nc.sync.dma_start(out=ctxf_sb[2 * hp + 1][:, :],
                                  in_=cc_out[hp][P:2 * P, :])

            def gather3(g):
                """hp3's AllGather, q-range piece g."""
                g0, g1 = G3[g]
                qc = slice(g0, g1)
                nc.sync.dma_start(out=cc_in3[g][:, :], in_=ctxn_sb[3][:, qc])
                if skip_cc:
                    nc.sync.dma_start(out=cc_out3[g][0:P, :],
                                      in_=cc_in3[g][:, :])
                    nc.sync.dma_start(out=cc_out3[g][P:2 * P, :],
                                      in_=cc_in3[g][:, :])
                else:
                    nc.gpsimd.collective_compute(
                        "AllGather",
                        mybir.AluOpType.bypass,
                        replica_groups=[[0, 1], [2, 3], [4, 5], [6, 7]],
                        ins=[cc_in3[g].opt()],
                        outs=[cc_out3[g].opt()],
                    )
                nc.sync.dma_start(out=ctxf_sb[6][:, qc], in_=cc_out3[g][0:P, :])
                nc.sync.dma_start(out=ctxf_sb[7][:, qc],
                                  in_=cc_out3[g][P:2 * P, :])

            # ---- emission schedule (program order ~ per-engine order) ----
            proj_unit(0, 0)
            v_unit(0)
            attention(0, 0)
            for j in (1, 2, 3):
                proj_unit(0, j)
                v_unit(j)
                attention(0, j)
            gather(0)
            proj_unit(1, 0)

            # o_units are stamped with tile_wait_until so the scheduler's
            # stage-1A sim (which models collectives as ~free) cannot hoist
            # them onto the PE queue ahead of ready attention work.
            attention(1, 0)
            proj_unit(1, 1)
            attention(1, 1)
            proj_unit(1, 2)
            with tc.tile_wait_until(0.200):
                o_unit(0, (0, 1, 2, 3, 4, 5, 6, 7))
            attention(1, 2)
            proj_unit(1, 3)
            with tc.tile_wait_until(0.215):
                o_unit(0, (8, 9, 10, 11, 12, 13, 14, 15))
            attention(1, 3)
            gather(1)
            proj_unit(2, 0)

            attention(2, 0)
            proj_unit(2, 1)
            attention(2, 1)
            proj_unit(2, 2)
            with tc.tile_wait_until(0.265):
                o_unit(1, (0, 1, 2, 3, 4, 5, 6, 7))
            attention(2, 2)
            proj_unit(2, 3)
            with tc.tile_wait_until(0.280):
                o_unit(1, (8, 9, 10, 11, 12, 13, 14, 15))
            attention(2, 3)
            gather(2)
            proj_unit(3, 0)

            attention(3, 0)
            proj_unit(3, 1)
            attention(3, 1)
            proj_unit(3, 2)
            gather3(0)
            with tc.tile_wait_until(0.330):
                o_unit(2, (0, 1, 2, 3, 4, 5, 6, 7))
            attention(3, 2)
            proj_unit(3, 3)
            gather3(1)
            with tc.tile_wait_until(0.345):
                o_unit(2, (8, 9, 10, 11, 12, 13, 14, 15))
            attention(3, 3)
            gather3(2)
            with tc.tile_wait_until(0.360):
                o_unit(3, (0, 1, 2, 3, 4, 5, 6, 7))   # cols [0:1024] via g3(0)
            with tc.tile_wait_until(0.370):
                o_unit(3, (8, 9, 10, 11))             # g3(1)
            with tc.tile_wait_until(0.380):
                o_unit(3, (12, 13, 14, 15))           # g3(2)

    nc.finalize()
    return nc


def kernel(payload, w_qkv, w_out, b_out):
    global LAST_EXEC_TIME_NS, _CACHED_NC
    payload = np.asarray(payload, dtype=np.float32)
    w_qkv = np.asarray(w_qkv, dtype=np.float32)
    w_out = np.asarray(w_out, dtype=np.float32)
    b_out = np.asarray(b_out, dtype=np.float32)

    bf = ml_dtypes.bfloat16
    # w_out rows permuted to match gathered ctx chunk order:
    # chunk 2*hp   = even core's head-pair hp -> rows [128hp, 128hp+128)
    # chunk 2*hp+1 = odd  core's head-pair hp -> rows [512+128hp, ...)
    row_perm = np.concatenate(
        [np.r_[128 * hp:128 * hp + 128, 512 + 128 * hp:512 + 128 * hp + 128]
         for hp in range(4)]
    )
    w_out_p = w_out[row_perm]

    in_maps = []
    for c in range(N_CORES):
        b, half = c // 2, c % 2
        cols = slice(half * DL, (half + 1) * DL)
        in_maps.append({
            "pt": np.ascontiguousarray(payload[b].T).astype(bf),
            "wq": np.ascontiguousarray(w_qkv[:, cols]).astype(bf),
            "wk": np.ascontiguousarray(w_qkv[:, D:][:, cols]).astype(bf),
            "wv": np.ascontiguousarray(w_qkv[:, 2 * D:][:, cols]).astype(bf),
            "wo": np.ascontiguousarray(w_out_p[:, cols]).astype(bf),
            "bo": np.ascontiguousarray(b_out[cols]).reshape(1, DL).astype(bf),
        })

    if _CACHED_NC is None:
        _CACHED_NC = _build()
    res = bass_utils.run_bass_kernel_spmd(
        _CACHED_NC, in_maps, core_ids=list(range(N_CORES))
    )
    LAST_EXEC_TIME_NS = res.exec_time_ns

    out = np.empty((B, S, D), dtype=np.float32)
    for c in range(N_CORES):
        b, half = c // 2, c % 2
        out[b, :, half * DL:(half + 1) * DL] = res.results[c]["out"]
    return out
